# revision 1
# baseline (speedup 1.0000x reference)
# Trainium2 Bass kernel for the Chebyshev-GCN GRU decoder (gnn_message_passing).
#
# Problem: B=16, N=2048, F=64, K=2 Chebyshev taps, T=8 decode steps.
#   per step: gates = cheb(L, [x, hx]) @ W_gate; r,u = sigmoid(gates)
#             cy = tanh(cheb(L, [x, r*hx]) @ W_upd); hy = u*hx + (1-u)*cy
#             yt = sigmoid(hy @ W_edge)
#
# Strategy (all math on device; host does only layout transforms + sharding):
#  - Data-parallel over batch: 8 cores x 2 batches each.
#  - x is re-fed every step, so all x-only terms are step-invariant:
#      L@x, x@W*_x, (L@x)@W*_x  ->  computed once on device ("Gconst"/"Cconst").
#    The per-step big matmuls shrink to L@hx and L@(r*hx)  (N x N x 64 each).
#  - Everything lives in "transposed" layout [c, n] with c = b*64 + f (128
#    partitions = 2 batches x 64 features), so the small (feature) matmuls
#    contract over partitions.  The two batches are kept independent in one
#    128-wide matmul by block-diagonal 128x128 weights (built on host).
#  - Big matmul orientation: out[c, n] += sum_m hx_nat[m, c] * L^T[m, n]:
#    stationary = hx in natural layout (16 tiles of [128m, 128c]), moving =
#    L^T (free dim 512 per instruction).  L^T stays fully SBUF-resident
#    (fits because the step-invariant gate consts are stored bf16).
#  - hx/r*hx needed both transposed (elementwise/small-mm) and natural
#    (stationary): regenerated each step with PE-mode transposes (16+16 tiles).
#  - Matmul dtype is a knob: float32r (single-pass fp32, full PE rate at free
#    dim >= 256) vs float32 (exact, 4 cycles/row).  PSUM accumulation is fp32
#    either way.
#
# The kernel() entry point takes FULL unsharded inputs and returns the FULL
# [T, B, N, F] output; it shards/reassembles on host.

import numpy as np
from contextlib import ExitStack

import concourse.bass as bass
import concourse.tile as tile
from concourse import bacc, mybir
from concourse.bass_utils import run_bass_kernel_spmd

F32 = mybir.dt.float32

B, N, F = 16, 2048, 64
T = 8
NCORES = 8
BL = B // NCORES          # batches per core (2)
C = BL * F                # 128 = partition width of transposed tensors
NT = N // 128             # 16 contraction tiles
NBLK = 4                  # n blocks per big matmul
BLK = N // NBLK           # 512 = free dim per matmul instruction
LT_RES = NT               # all 16 L^T row-blocks stay SBUF-resident

# Matmul-operand dtype knob.  float32r = single-pass fp32 matmul (full PE
# rate at free dim >= 256, reduced multiply precision); float32 = exact,
# 4 cycles/row.  walrus requires fp32r operands to be *produced* as fp32r,
# so every tensor feeding a matmul is declared MM_DT end-to-end (same bytes
# as fp32 in memory; numpy side stays float32).
MM_DT = mybir.dt.float32r
BF16 = mybir.dt.bfloat16   # storage dtype of the step-invariant gate consts

W_NAMES = [
    "wh0r", "wh0u", "wh1r", "wh1u",   # gates, hx / L@hx terms (r and u halves)
    "wx0r", "wx0u", "wx1r", "wx1u",   # gates, x / L@x terms (precompute)
    "whc0", "whc1",                   # candidate, r*hx / L@(r*hx) terms
    "wxc0", "wxc1",                   # candidate, x / L@x terms (precompute)
    "we",                             # edge output projection
]
B_NAMES = ["bgr", "bgu", "bcc", "bee"]


def _emit(ctx: ExitStack, tc: tile.TileContext, d):
    """Emit the per-core program.  `d` maps dram tensor name -> AP."""
    nc = tc.nc
    AF = mybir.ActivationFunctionType

    consts = ctx.enter_context(tc.tile_pool(name="consts", bufs=1))
    work = ctx.enter_context(tc.tile_pool(name="work", bufs=2))
    tmp3 = ctx.enter_context(tc.tile_pool(name="tmp", bufs=3))
    big_ps = ctx.enter_context(tc.tile_pool(name="bigps", bufs=2, space="PSUM"))
    small_ps = ctx.enter_context(tc.tile_pool(name="smallps", bufs=4, space="PSUM"))
    tr_ps = ctx.enter_context(tc.tile_pool(name="trps", bufs=2, space="PSUM"))

    # ---- static loads -------------------------------------------------
    # all 13 weights + identity packed into one DMA; biases in another
    wpack = consts.tile([128, 14 * 128], MM_DT, tag="wpack")
    nc.sync.dma_start(wpack[:], d["wpack"][:, :])
    w = {name: wpack[:, i * 128:(i + 1) * 128]
         for i, name in enumerate(W_NAMES)}
    ident = wpack[:, 13 * 128:14 * 128]
    bpack = consts.tile([128, len(B_NAMES)], F32, tag="bpack")
    nc.sync.dma_start(bpack[:], d["bpack"][:, :])
    bias = {name: bpack[:, j:j + 1] for j, name in enumerate(B_NAMES)}

    # stationary buffer: holds x_nat, then alternates hx_nat -> rh_nat per step
    s_sb = consts.tile([128, N], MM_DT, tag="s")
    try:
        nc.sync.dma_start(s_sb[:].rearrange("p (a c) -> p a c", c=128),
                          d["xnat"].rearrange("(a p) c -> p a c", p=128))
    except Exception:
        for mi in range(NT):
            nc.sync.dma_start(s_sb[:, mi * 128:(mi + 1) * 128],
                              d["xnat"][mi * 128:(mi + 1) * 128, :])
    # xt shares its slot with uT (dead after precompute)
    xt_sb = consts.tile([128, N], MM_DT, tag="xt_u")
    nc.sync.dma_start(xt_sb[:], d["xt"][:, :])

    # L^T load is the long pole at kernel start (~14MB): emitted LAST among
    # the static loads (the dynamic-DGE ring issues in order, so anything
    # after it would stall ~40us) and as 256KB chunks in the order the
    # first big matmuls consume them, so PE starts once chunk (0,0) lands.
    lt_sb = consts.tile([128, LT_RES * N], MM_DT, tag="ltsb")
    for blk in range(NBLK):
        for mi in range(LT_RES):
            nc.sync.dma_start(
                lt_sb[:, mi * N + blk * BLK: mi * N + (blk + 1) * BLK],
                d["lt"][mi * 128:(mi + 1) * 128, blk * BLK:(blk + 1) * BLK])

    hxbuf = [consts.tile([128, N], MM_DT, tag=f"hxT{i}", name=f"hxT{i}")
             for i in range(2)]
    rhT = consts.tile([128, N], MM_DT, tag="rhT")
    grc = consts.tile([128, N], BF16, tag="grc")
    guc = consts.tile([128, N], BF16, tag="guc")
    ccc = consts.tile([128, N], BF16, tag="ccc")
    # full-width landing buffer for (L @ stationary)^T; shared by the
    # precompute LxT, phase-A LhT and phase-B LrhT (disjoint lifetimes)
    lxh = consts.tile([128, N], MM_DT, tag="lxh")

    def lt_rhs(mi, blk):
        """[128, BLK] moving-operand slice of L^T for row-block mi, n-block blk."""
        return lt_sb[:, mi * N + blk * BLK: mi * N + (blk + 1) * BLK]

    def big_mm(blk):
        """psum[c, n_blk] = sum_m s_sb[m, c] * L^T[m, n_blk]  (16-tile accum)."""
        ps = big_ps.tile([128, BLK], F32, tag="big")
        for mi in range(NT):
            nc.tensor.matmul(
                ps[:],
                s_sb[:, mi * 128:(mi + 1) * 128],
                lt_rhs(mi, blk),
                start=(mi == 0), stop=(mi == NT - 1))
        return ps

    def small_mm(pairs, const_ap=None):
        """psum = sum_i w_i.T @ rhs_i; then += const_ap in place (DVE)."""
        ps = small_ps.tile([128, BLK], F32, tag="small")
        for i, (wt, rhs) in enumerate(pairs):
            nc.tensor.matmul(ps[:], wt[:], rhs,
                             start=(i == 0), stop=(i == len(pairs) - 1))
        if const_ap is not None:
            nc.vector.tensor_add(ps[:], ps[:], const_ap)
        return ps

    def transpose_to_s(src, blk):
        """PE-transpose 4 [128,128] tiles of src n-block blk into s_sb."""
        pt = tr_ps.tile([128, BLK], MM_DT, tag="tr")
        for j in range(4):
            mi = blk * 4 + j
            nc.tensor.transpose(pt[:, j * 128:(j + 1) * 128],
                                src[:, mi * 128:(mi + 1) * 128], ident[:])
        nc.vector.tensor_copy(s_sb[:, blk * BLK:(blk + 1) * BLK], pt[:])

    def nb(ap, blk):
        return ap[:, blk * BLK:(blk + 1) * BLK]

    # ---- precompute: LxT = (L@x)^T, then the step-invariant gate/cand consts
    for blk in range(NBLK):
        ps = big_mm(blk)                       # s_sb holds x_nat here
        nc.vector.tensor_copy(nb(lxh, blk), ps[:])
    for blk in range(NBLK):
        for wa, wb, bi, dst in (("wx0r", "wx1r", "bgr", grc),
                                ("wx0u", "wx1u", "bgu", guc),
                                ("wxc0", "wxc1", "bcc", ccc)):
            psg = small_mm([(w[wa], nb(xt_sb, blk)), (w[wb], nb(lxh, blk))])
            nc.scalar.activation(nb(dst, blk), psg[:], AF.Identity, bias=bias[bi][:])

    uT = consts.tile([128, N], F32, tag="xt_u")   # reuses xt slot

    def emit_out(t, hyT, blk):
        """yt = sigmoid(W_edge.T @ hy + b_edge) -> DRAM out[t]."""
        ps = small_mm([(w["we"], nb(hyT, blk))])
        ytt = tmp3.tile([128, BLK], F32, tag="tmp")
        nc.scalar.activation(ytt[:], ps[:], AF.Sigmoid, bias=bias["bee"][:])
        nc.sync.dma_start(d["out"][t, :, blk * BLK:(blk + 1) * BLK], ytt[:])

    # ---- step 0 (hx == 0: no big matmuls, r unused) -------------------
    hyT = hxbuf[1]
    for blk in range(NBLK):
        nc.scalar.activation(nb(uT, blk), nb(guc, blk), AF.Sigmoid)
        cyt = work.tile([128, BLK], F32, tag="cyt")
        nc.scalar.activation(cyt[:], nb(ccc, blk), AF.Tanh)
        e = tmp3.tile([128, BLK], F32, tag="tmp")
        nc.vector.tensor_mul(e[:], nb(uT, blk), cyt[:])
        nc.vector.tensor_sub(nb(hyT, blk), cyt[:], e[:])   # hy0 = (1-u)*cy
        emit_out(0, hyT, blk)
        transpose_to_s(hyT, blk)

    # ---- steps 1..T-1 -------------------------------------------------
    for t in range(1, T):
        hxT, hyT = hxbuf[t % 2], hxbuf[(t + 1) % 2]
        # phase A1: Lh = (L@hx)^T for ALL blocks (s_sb must stay hx_nat
        # until every big matmul has read it)
        for blk in range(NBLK):
            ps = big_mm(blk)                   # s_sb holds hx_nat
            nc.vector.tensor_copy(nb(lxh, blk), ps[:])
        # phase A2: r,u; rh = r*hx; transpose rh -> s_sb.
        # Emission order = engine-queue order, so everything on the
        # r-critical path (r matmuls, r const-adds, r sigmoids, rh muls,
        # transposes) is emitted before the u-gate work, which is only
        # needed late in phase B2.
        psrs = [small_mm([(w["wh0r"], nb(hxT, blk)),
                          (w["wh1r"], nb(lxh, blk))],
                         const_ap=nb(grc, blk)) for blk in range(NBLK)]
        psus = [small_mm([(w["wh0u"], nb(hxT, blk)),
                          (w["wh1u"], nb(lxh, blk))]) for blk in range(NBLK)]
        for blk in range(NBLK):
            nc.scalar.activation(nb(rhT, blk), psrs[blk][:], AF.Sigmoid)
        for blk in range(NBLK):
            nc.vector.tensor_mul(nb(rhT, blk), nb(rhT, blk), nb(hxT, blk))
        for blk in range(NBLK):
            transpose_to_s(rhT, blk)
        for blk in range(NBLK):
            nc.vector.tensor_add(psus[blk][:], psus[blk][:], nb(guc, blk))
            nc.scalar.activation(nb(uT, blk), psus[blk][:], AF.Sigmoid)
        # off-critical blend prep (runs while PE does phase B1): the hyT
        # ping-pong buffer is dead from here on (B2 no longer reads hx), so
        # stage W = u*hx directly in it; then uT := (1-u) in place.  The
        # post-tanh chain at each step boundary becomes tanh -> mul -> add.
        for blk in range(NBLK):
            nc.vector.tensor_mul(nb(hyT, blk), nb(uT, blk), nb(hxT, blk))
        for blk in range(NBLK):
            nc.vector.tensor_scalar(nb(uT, blk), nb(uT, blk), -1.0, 1.0,
                                    op0=mybir.AluOpType.mult,
                                    op1=mybir.AluOpType.add)
        # phase B1: Lrh = (L@(r*hx))^T for ALL blocks
        for blk in range(NBLK):
            ps = big_mm(blk)                   # s_sb holds rh_nat
            nc.scalar.copy(nb(lxh, blk), ps[:])
        # phase B2, software-pipelined: cy matmul groups are interleaved
        # with the previous block's tanh->mul->add tail so the DVE const-
        # adds never queue ahead of the critical pp/hy ops
        pscs = [None] * NBLK

        def b2_tail(blk):
            cyt = work.tile([128, BLK], F32, tag="cyt")
            nc.scalar.activation(cyt[:], pscs[blk][:], AF.Tanh)
            pp = tmp3.tile([128, BLK], F32, tag="tmp")
            nc.vector.tensor_mul(pp[:], nb(uT, blk), cyt[:])   # (1-u)*cy
            nc.vector.tensor_add(nb(hyT, blk), nb(hyT, blk), pp[:])
            if t < T - 1:
                transpose_to_s(hyT, blk)
            emit_out(t, hyT, blk)

        for blk in range(2):
            pscs[blk] = small_mm([(w["whc0"], nb(rhT, blk)),
                                  (w["whc1"], nb(lxh, blk))],
                                 const_ap=nb(ccc, blk))
        b2_tail(0)
        pscs[2] = small_mm([(w["whc0"], nb(rhT, 2)),
                            (w["whc1"], nb(lxh, 2))], const_ap=nb(ccc, 2))
        b2_tail(1)
        pscs[3] = small_mm([(w["whc0"], nb(rhT, 3)),
                            (w["whc1"], nb(lxh, 3))], const_ap=nb(ccc, 3))
        b2_tail(2)
        b2_tail(3)


_BUILT = {}


def _build():
    if "nc" in _BUILT:
        return _BUILT["nc"]
    nc = bacc.Bacc("TRN2", target_bir_lowering=False, debug=False)
    d = {}
    d["lt"] = nc.dram_tensor("lt", [N, N], MM_DT, kind="ExternalInput").ap()
    d["xnat"] = nc.dram_tensor("xnat", [N, C], MM_DT, kind="ExternalInput").ap()
    d["xt"] = nc.dram_tensor("xt", [C, N], MM_DT, kind="ExternalInput").ap()
    d["wpack"] = nc.dram_tensor("wpack", [128, 14 * 128], MM_DT,
                                kind="ExternalInput").ap()
    d["bpack"] = nc.dram_tensor("bpack", [128, len(B_NAMES)], F32,
                                kind="ExternalInput").ap()
    d["out"] = nc.dram_tensor("out", [T, C, N], F32, kind="ExternalOutput").ap()

    with tile.TileContext(nc) as tc, ExitStack() as ctx:
        _emit(ctx, tc, d)
    nc.compile()
    _BUILT["nc"] = nc
    return nc


def _bd(m):
    """[64,64] -> block-diagonal [128,128] (two independent batches)."""
    z = np.zeros((128, 128), np.float32)
    z[:64, :64] = m
    z[64:, 64:] = m
    return z


def make_in_maps(inputs_edge, L_tilde, W_gate, b_gate, W_upd, b_upd,
                 W_edge, b_edge):
    """Host-side layout transforms + per-core sharding (no math)."""
    x = np.asarray(inputs_edge, np.float32)
    L = np.asarray(L_tilde, np.float32)
    Wg0, Wg1 = np.asarray(W_gate[0], np.float32), np.asarray(W_gate[1], np.float32)
    Wu0, Wu1 = np.asarray(W_upd[0], np.float32), np.asarray(W_upd[1], np.float32)
    We = np.asarray(W_edge, np.float32)
    bg = np.asarray(b_gate, np.float32)
    bu = np.asarray(b_upd, np.float32)
    be = np.asarray(b_edge, np.float32)

    wmats = {
        "wh0r": _bd(Wg0[64:, :64]), "wh0u": _bd(Wg0[64:, 64:]),
        "wh1r": _bd(Wg1[64:, :64]), "wh1u": _bd(Wg1[64:, 64:]),
        "wx0r": _bd(Wg0[:64, :64]), "wx0u": _bd(Wg0[:64, 64:]),
        "wx1r": _bd(Wg1[:64, :64]), "wx1u": _bd(Wg1[:64, 64:]),
        "whc0": _bd(Wu0[64:]), "whc1": _bd(Wu1[64:]),
        "wxc0": _bd(Wu0[:64]), "wxc1": _bd(Wu1[:64]),
        "we": _bd(We),
    }
    wpack = np.concatenate([wmats[n] for n in W_NAMES]
                           + [np.eye(128, dtype=np.float32)], axis=1)
    bpack = np.stack([np.tile(bg[:64], 2), np.tile(bg[64:], 2),
                      np.tile(bu, 2), np.tile(be, 2)], axis=1)
    shared = {
        "lt": np.ascontiguousarray(L.T),
        "wpack": np.ascontiguousarray(wpack),
        "bpack": np.ascontiguousarray(bpack.astype(np.float32)),
    }
    in_maps = []
    for core in range(NCORES):
        xs = x[core * BL:(core + 1) * BL]                    # [BL, N, F]
        m = dict(shared)
        m["xnat"] = np.ascontiguousarray(xs.transpose(1, 0, 2).reshape(N, C))
        m["xt"] = np.ascontiguousarray(xs.transpose(0, 2, 1).reshape(C, N))
        in_maps.append(m)
    return in_maps


def unshard(core_outs):
    """[NCORES][T, C, N] -> [T, B, N, F]"""
    arr = np.stack(core_outs)                                # [8, T, 128, N]
    return np.ascontiguousarray(
        arr.reshape(NCORES, T, BL, F, N)
           .transpose(1, 0, 2, 4, 3)
           .reshape(T, B, N, F).astype(np.float32))


def run(in_maps, **kw):
    nc = _build()
    return run_bass_kernel_spmd(nc, in_maps, list(range(NCORES)), **kw)


def kernel(inputs_edge, L_tilde, W_gate, b_gate, W_upd, b_upd, W_edge, b_edge):
    in_maps = make_in_maps(inputs_edge, L_tilde, W_gate, b_gate,
                           W_upd, b_upd, W_edge, b_edge)
    res = run(in_maps)
    return unshard([res.results[c]["out"] for c in range(NCORES)])



# revision 28
# speedup vs baseline: 1.7074x; 1.7074x over previous
# Trainium2 Bass kernel for the Chebyshev-GCN GRU decoder (gnn_message_passing).
#
# Problem: B=16, N=2048, F=64, K=2 Chebyshev taps, T=8 decode steps.
#   per step: gates = cheb(L, [x, hx]) @ W_gate; r,u = sigmoid(gates)
#             cy = tanh(cheb(L, [x, r*hx]) @ W_upd); hy = u*hx + (1-u)*cy
#             yt = sigmoid(hy @ W_edge)
#
# v2 strategy (fp8 DoubleRow everywhere on the per-step critical path):
#  - Data-parallel over batch: 8 cores x 2 batches each; transposed layout
#    [c, n] with c = b*64+f (128 partitions), block-diagonal 128x128 weights.
#  - x-only terms are step-invariant: computed ONCE at startup into gate
#    consts (grc/guc/ccc), stored as fp8 hi+lo pairs at x32 scale so the
#    per-step const-add rides the fp8 DoubleRow matmul path.
#  - Precompute L@x runs in fp8 hi/lo split form (L8@xh + L8@xl + Lres@xh,
#    all DoubleRow) which matches bf16 accuracy without an 8MB bf16 L load.
#  - Per-step big matmuls L@hx, L@(r*hx): fp8e4m3 DoubleRow (2 k-tiles of
#    128 per instruction, 0.5 cyc/row): L^T stored x32 fp8, activations x8
#    fp8; psum carries x256, drained x(1/32) to fp8 x8 moving operands.
#  - Per-step feature matmuls: fp8 DoubleRow with moving pairs
#    (hxT8|lxh8) / (rhT8|Lrh8) and weight pairs (W0|W1)x4; gate consts
#    enter the same psum group via an (I|I) @ (hi|lo) DoubleRow pair.
#    psum = 32x gates; Act sigmoid/tanh unwinds with scale=1/32.
#  - Edge output projection stays bf16 (fp8 there costs too much accuracy).
#  - Elementwise in bf16 on DVE (4x TensorScalarPtr perf mode) + GPSIMD for
#    the SBUF->SBUF fp8 casts (no PSUM port on gpsimd); psum drains split
#    DVE/Act.
#
# kernel() takes FULL unsharded inputs, returns FULL [T, B, N, F] output.

import numpy as np
import ml_dtypes
from contextlib import ExitStack

import concourse.bass as bass
import concourse.tile as tile
from concourse import bacc, mybir
from concourse.bass_utils import run_bass_kernel_spmd

F32 = mybir.dt.float32
BF16 = mybir.dt.bfloat16
FP8 = mybir.dt.float8e4
U32 = mybir.dt.uint32
DR = mybir.MatmulPerfMode.DoubleRow
NPF8 = ml_dtypes.float8_e4m3
NPBF = ml_dtypes.bfloat16

B, N, F = 16, 2048, 64
T = 8
NCORES = 8
BL = B // NCORES          # batches per core (2)
C = BL * F                # 128 partitions in transposed layout
NT = N // 128             # 16 contraction tiles
NBLK = 4
BLK = N // NBLK           # 512

MU = mybir.AluOpType.mult
AD = mybir.AluOpType.add
SU = mybir.AluOpType.subtract


def bw(blk):
    return slice(blk * BLK, (blk + 1) * BLK)


def _emit(ctx: ExitStack, tc: tile.TileContext, d):
    nc = tc.nc
    AF = mybir.ActivationFunctionType

    consts = ctx.enter_context(tc.tile_pool(name="consts", bufs=1))
    big_ps = ctx.enter_context(tc.tile_pool(name="bigps", bufs=2, space="PSUM"))
    ru_ps = ctx.enter_context(tc.tile_pool(name="rups", bufs=2, space="PSUM"))
    tr_ps = ctx.enter_context(tc.tile_pool(name="trps", bufs=1, space="PSUM"))

    # ---- persistent SBUF tiles --------------------------------------
    w8 = consts.tile([128, 2, 512], FP8, tag="w8")
    wb = consts.tile([128, 8 * 128], BF16, tag="wb")
    bias = consts.tile([128, 6], F32, tag="bias")
    lt8 = consts.tile([128, NT, N], FP8, tag="lt8")
    s_sb = consts.tile([128, NT, 128], FP8, tag="s")
    mvA = consts.tile([128, 2, N], FP8, tag="mvA")     # hxT8 | lxh8
    mvB = consts.tile([128, 2, N], FP8, tag="mvB")     # rhT8 | Lrh8
    gc = consts.tile([128, 2, N], FP8, tag="gc")       # hi | lo (x32)
    gu = consts.tile([128, 2, N], FP8, tag="gu")
    cc = consts.tile([128, 2, N], FP8, tag="cc")
    hx_t = [consts.tile([128, N], BF16, tag=f"hx{i}", name=f"hx{i}")
            for i in range(2)]
    ruT = consts.tile([128, 2, N], BF16, tag="ruT")    # r | u
    cyt = consts.tile([128, N], BF16, tag="cyt")
    qtmp = consts.tile([128, N], BF16, tag="qtmp")
    ytt = consts.tile([128, N], F32, tag="ytt")

    ident8 = w8[:, 0, 384:512]
    ipair = w8[:, :, 384:512]
    w_gr = w8[:, :, 0:128]
    w_gu = w8[:, :, 128:256]
    w_cc = w8[:, :, 256:384]
    wbm = {k: wb[:, i * 128:(i + 1) * 128]
           for i, k in enumerate(["wx0r", "wx0u", "wxc0",
                                  "wx1r", "wx1u", "wxc1", "we", "identb"])}
    b_gr32 = bias[:, 0:1]
    b_gu32 = bias[:, 1:2]
    b_cc32 = bias[:, 2:3]
    b_ngu = bias[:, 3:4]
    b_cct = bias[:, 4:5]
    b_ee = bias[:, 5:6]
    rT = ruT[:, 0, :]
    uT = ruT[:, 1, :]

    # ---- static loads (small first; L chunks in consumption order) --
    nc.sync.dma_start(w8[:], d["w8"][:, :, :])
    nc.sync.dma_start(wb[:], d["wb"][:, :])
    nc.sync.dma_start(bias[:], d["bias"][:, :])

    def emit_mms(hyT, half):
        """half of y[t]'s projection: 2 blocks into one [128,1024] psum."""
        ep = ru_ps.tile([128, 2 * BLK], F32, tag="ru", name="ep")
        for j in range(2):
            blk = 2 * half + j
            nc.tensor.matmul(ep[:, j * BLK:(j + 1) * BLK], wbm["we"],
                             hyT[:, bw(blk)], start=True, stop=True)
        return ep

    def emit_act(t, half, ep):
        hw2 = slice(half * 2 * BLK, (half + 1) * 2 * BLK)
        nc.scalar.activation(ytt[:, hw2], ep[:], AF.Sigmoid,
                             bias=b_ee, scale=1.0)
        nc.sync.dma_start(d["out"][t, :, hw2], ytt[:, hw2])

    # ---- precompute -------------------------------------------------
    with tc.tile_pool(name="pre", bufs=1) as pre:
        lres = pre.tile([128, NT, N], FP8, tag="lres")
        xq = pre.tile([128, 2, NT, 128], FP8, tag="xq")   # xh | xl planes
        xt = pre.tile([128, N], BF16, tag="xt")
        lxp = pre.tile([128, N], BF16, tag="lxp")

        nc.sync.dma_start(
            xq[:], d["xq"].rearrange("l (a p) c -> p l a c", p=128))
        nc.sync.dma_start(xt[:], d["xt"][:, :])
        dlt = d["lt8"].rearrange("(a p) c -> p a c", p=128)
        dlr = d["lres"].rearrange("(a p) c -> p a c", p=128)
        for blk in range(NBLK):
            nc.sync.dma_start(lt8[:, :, bw(blk)], dlt[:, :, bw(blk)])
            nc.sync.dma_start(lres[:, :, bw(blk)], dlr[:, :, bw(blk)])

        # Lx = L8@(xh+xl) + Lres@xh   (all DoubleRow, psum x256)
        for blk in range(NBLK):
            ps = big_ps.tile([128, BLK], F32, tag="big")
            k = 0
            for plane, lsb in ((0, lt8), (1, lt8), (0, lres)):
                for m in range(8):
                    nc.tensor.matmul(
                        ps[:], xq[:, plane, 2 * m:2 * m + 2, :],
                        lsb[:, 2 * m:2 * m + 2, bw(blk)],
                        start=(k == 0), stop=(k == 23), perf_mode=DR)
                    k += 1
            nc.vector.tensor_scalar(lxp[:, bw(blk)], ps[:],
                                    1.0 / 256.0, None, op0=MU)

        # gate/cand consts (psum x32 via x32 bf16 weights) + step-0 tail
        trt0 = tr_ps.tile([128, 4 * BLK], BF16, tag="tr")
        for blk in range(NBLK):
            pg = ru_ps.tile([128, 2 * BLK], F32, tag="ru")
            nc.tensor.matmul(pg[:, 0:BLK], wbm["wx0r"], xt[:, bw(blk)],
                             start=True, stop=False)
            nc.tensor.matmul(pg[:, 0:BLK], wbm["wx1r"], lxp[:, bw(blk)],
                             start=False, stop=True)
            nc.tensor.matmul(pg[:, BLK:], wbm["wx0u"], xt[:, bw(blk)],
                             start=True, stop=False)
            nc.tensor.matmul(pg[:, BLK:], wbm["wx1u"], lxp[:, bw(blk)],
                             start=False, stop=True)
            pc_t = ru_ps.tile([128, 2 * BLK], F32, tag="ru")
            pc = pc_t[:, 0:BLK]
            nc.tensor.matmul(pc, wbm["wxc0"], xt[:, bw(blk)],
                             start=True, stop=False)
            nc.tensor.matmul(pc, wbm["wxc1"], lxp[:, bw(blk)],
                             start=False, stop=True)
            # hi/lo splits (consts include bias*32)
            nc.scalar.activation(gc[:, 0, bw(blk)], pg[:, 0:BLK],
                                 AF.Identity, bias=b_gr32, scale=1.0)
            nc.scalar.activation(gu[:, 0, bw(blk)], pg[:, BLK:],
                                 AF.Identity, bias=b_gu32, scale=1.0)
            nc.scalar.activation(cc[:, 0, bw(blk)], pc,
                                 AF.Identity, bias=b_cc32, scale=1.0)
            nc.vector.scalar_tensor_tensor(gc[:, 1, bw(blk)], pg[:, 0:BLK],
                                           b_gr32, gc[:, 0, bw(blk)],
                                           op0=AD, op1=SU)
            nc.vector.scalar_tensor_tensor(gu[:, 1, bw(blk)], pg[:, BLK:],
                                           b_gu32, gu[:, 0, bw(blk)],
                                           op0=AD, op1=SU)
            nc.vector.scalar_tensor_tensor(cc[:, 1, bw(blk)], pc,
                                           b_cc32, cc[:, 0, bw(blk)],
                                           op0=AD, op1=SU)
            # step 0: ucompl = sigmoid(-gu0), cy0 = tanh(cc0),
            # hy0 = ucompl*cy0  (psums are x32, biases not yet applied)
            nc.scalar.activation(uT[:, bw(blk)], pg[:, BLK:], AF.Sigmoid,
                                 bias=b_ngu, scale=-1.0 / 32.0)
            nc.scalar.activation(cyt[:, bw(blk)], pc, AF.Tanh,
                                 bias=b_cct, scale=1.0 / 32.0)
            nc.vector.tensor_mul(hx_t[1][:, bw(blk)], uT[:, bw(blk)],
                                 cyt[:, bw(blk)])
            nc.scalar.mul(mvA[:, 0, bw(blk)], hx_t[1][:, bw(blk)], 8.0)
            for j in range(4):
                ti = blk * 4 + j
                nc.tensor.transpose(trt0[:, ti * 128:(ti + 1) * 128],
                                    hx_t[1][:, ti * 128:(ti + 1) * 128],
                                    wbm["identb"])
        for h in range(2):
            nc.vector.tensor_scalar(
                s_sb[:, h * 8:h * 8 + 8, :].rearrange("p a c -> p (a c)"),
                trt0[:, h * 1024:(h + 1) * 1024], 8.0, None, op0=MU)

    # ---- steps 1..T-1 ----------------------------------------------
    for t in range(1, T):
        hxT, hyT = hx_t[t % 2], hx_t[(t + 1) % 2]
        # Emission builds per-engine queues so no ready work sits behind
        # blocked work.  Big matmuls: per block, k-pairs 0-3 then 4-7 as
        # separate half-accumulations interleaved across a block pair.
        def half(ps, blk, mr):
            for m in mr:
                nc.tensor.matmul(ps[:], s_sb[:, 2 * m:2 * m + 2, :],
                                 lt8[:, 2 * m:2 * m + 2, bw(blk)],
                                 start=(m == 0), stop=(m == 7), perf_mode=DR)

        lo, hi = range(0, 4), range(4, 8)

        # --- A phase -------------------------------------------------
        # A1 group 0 (blocks 0,1) + gates01 + r-sigs, then group 1.
        pra = [None, None]
        pua = [None, None]

        def gates_blk(h, j):
            if j == 0:
                pra[h] = ru_ps.tile([128, 2 * BLK], F32, tag="ru", name="pra")
                pua[h] = ru_ps.tile([128, 2 * BLK], F32, tag="ru", name="pua")
            blk = 2 * h + j
            js = slice(j * BLK, (j + 1) * BLK)
            for pg, w_g, gk in ((pra[h], w_gr, gc), (pua[h], w_gu, gu)):
                nc.tensor.matmul(pg[:, js], w_g, mvA[:, :, bw(blk)],
                                 start=True, stop=False, perf_mode=DR)
                nc.tensor.matmul(pg[:, js], ipair, gk[:, :, bw(blk)],
                                 start=False, stop=True, perf_mode=DR)

        def r_sig(blk):
            nc.scalar.activation(rT[:, bw(blk)],
                                 pra[blk // 2][:, (blk % 2) * BLK:
                                               (blk % 2 + 1) * BLK],
                                 AF.Sigmoid, scale=1.0 / 32.0)
            nc.vector.scalar_tensor_tensor(mvB[:, 0, bw(blk)],
                                           rT[:, bw(blk)], 8.0,
                                           hxT[:, bw(blk)], op0=MU, op1=MU)

        bigp = {}
        for g in range(2):
            b0, b1 = 2 * g, 2 * g + 1
            ps0 = big_ps.tile([128, BLK], F32, tag="big", name="ps0")
            ps1 = big_ps.tile([128, BLK], F32, tag="big", name="ps1")
            half(ps0, b0, lo)
            half(ps1, b1, lo)
            half(ps0, b0, hi)
            half(ps1, b1, hi)
            for ps, blk in ((ps0, b0), (ps1, b1)):
                nc.vector.tensor_scalar(mvA[:, 1, bw(blk)], ps[:],
                                        1.0 / 32.0, None, op0=MU)
            gates_blk(g, 0)
            gates_blk(g, 1)
            r_sig(2 * g)
            r_sig(2 * g + 1)

        def u_sig(blk):
            nc.scalar.activation(uT[:, bw(blk)],
                                 pua[blk // 2][:, (blk % 2) * BLK:
                                               (blk % 2 + 1) * BLK],
                                 AF.Sigmoid, scale=1.0 / 32.0)

        for blk in range(NBLK):
            u_sig(blk)

        # rh transposes (per block) with punned u32 drains; interleaved
        # with B1 half-accumulations so B1 never waits on late blocks.
        trt = tr_ps.tile([128, 4 * BLK], BF16, tag="tr")

        def tr_rh(blk):
            for j in range(4):
                ti = blk * 4 + j
                nc.tensor.transpose(trt[:, ti * 128:(ti + 1) * 128],
                                    rT[:, ti * 128:(ti + 1) * 128],
                                    wbm["identb"])
            sl = (s_sb[:, blk * 4:blk * 4 + 4, :]
                  .rearrange("p a c -> p (a c)"))
            nc.vector.tensor_mul(sl, sl, trt[:, bw(blk)])

        tr_rh(0)
        tr_rh(1)
        pb0 = big_ps.tile([128, BLK], F32, tag="big", name="pb0")
        pb1 = big_ps.tile([128, BLK], F32, tag="big", name="pb1")
        half(pb0, 0, lo)
        half(pb1, 1, lo)
        tr_rh(2)
        tr_rh(3)
        half(pb0, 0, hi)
        half(pb1, 1, hi)
        for ps, blk in ((pb0, 0), (pb1, 1)):
            nc.vector.tensor_scalar(mvB[:, 1, bw(blk)], ps[:],
                                    1.0 / 32.0, None, op0=MU)
        # z = 1-u, W = u*hx ride the DVE hole while B1 group 1 runs
        for h in range(2):
            hs = slice(h * 1024, (h + 1) * 1024)
            nc.vector.tensor_scalar(qtmp[:, hs], uT[:, hs],
                                    -1.0, 1.0, op0=MU, op1=AD)
            nc.vector.tensor_mul(hyT[:, hs], uT[:, hs], hxT[:, hs])
        pb2 = big_ps.tile([128, BLK], F32, tag="big", name="pb2")
        pb3 = big_ps.tile([128, BLK], F32, tag="big", name="pb3")
        half(pb2, 2, lo)
        half(pb3, 3, lo)
        half(pb2, 2, hi)
        half(pb3, 3, hi)
        for ps, blk in ((pb2, 2), (pb3, 3)):
            nc.scalar.mul(mvB[:, 1, bw(blk)], ps[:], 1.0 / 32.0)

        # --- B tail: per-block cand -> tanh -> blend -> cast -> transpose
        trt2 = (tr_ps.tile([128, 4 * BLK], BF16, tag="tr", name="trt2")
                if t < T - 1 else None)
        pcs = [None, None]

        def cand_blk(blk):
            h, j = blk // 2, blk % 2
            if j == 0:
                pcs[h] = ru_ps.tile([128, 2 * BLK], F32, tag="ru", name="pc")
            js = slice(j * BLK, (j + 1) * BLK)
            nc.tensor.matmul(pcs[h][:, js], w_cc, mvB[:, :, bw(blk)],
                             start=True, stop=False, perf_mode=DR)
            nc.tensor.matmul(pcs[h][:, js], ipair, cc[:, :, bw(blk)],
                             start=False, stop=True, perf_mode=DR)

        def tanh_blk(blk):
            nc.scalar.activation(cyt[:, bw(blk)],
                                 pcs[blk // 2][:, (blk % 2) * BLK:
                                               (blk % 2 + 1) * BLK],
                                 AF.Tanh, scale=1.0 / 32.0)

        def blend_blk(blk):
            nc.vector.tensor_mul(qtmp[:, bw(blk)], qtmp[:, bw(blk)],
                                 cyt[:, bw(blk)])
            nc.vector.tensor_add(hyT[:, bw(blk)], hyT[:, bw(blk)],
                                 qtmp[:, bw(blk)])

        def cast_blk(blk):
            if blk % 2 == 0:
                nc.scalar.mul(mvA[:, 0, bw(blk)], hyT[:, bw(blk)], 8.0)
            else:
                nc.vector.tensor_scalar(mvA[:, 0, bw(blk)], hyT[:, bw(blk)],
                                        8.0, None, op0=MU)

        def tr_hy(blk):
            for j in range(4):
                ti = blk * 4 + j
                nc.tensor.transpose(trt2[:, ti * 128:(ti + 1) * 128],
                                    hyT[:, ti * 128:(ti + 1) * 128],
                                    wbm["identb"])
            dsl = (s_sb[:, blk * 4:blk * 4 + 4, :]
                   .rearrange("p a c -> p (a c)"))
            if blk % 2 == 0:
                nc.vector.tensor_scalar(dsl, trt2[:, bw(blk)], 8.0, None,
                                        op0=MU)
            else:
                nc.scalar.mul(dsl, trt2[:, bw(blk)], 8.0)

        cand_blk(0)
        cand_blk(1)
        tanh_blk(0)
        cand_blk(2)
        tanh_blk(1)
        cand_blk(3)
        blend_blk(0)
        tanh_blk(2)
        blend_blk(1)
        cast_blk(1)
        tanh_blk(3)
        blend_blk(2)
        cast_blk(0)
        blend_blk(3)
        cast_blk(2)
        cast_blk(3)
        if t < T - 1:
            tr_hy(0)
            tr_hy(1)
        ep0 = emit_mms(hxT, 0)
        if t < T - 1:
            tr_hy(2)
            tr_hy(3)
        ep1 = emit_mms(hxT, 1)
        emit_act(t - 1, 0, ep0)
        emit_act(t - 1, 1, ep1)
        if t == T - 1:
            epa = emit_mms(hyT, 0)
            emit_act(t, 0, epa)
            epb = emit_mms(hyT, 1)
            emit_act(t, 1, epb)


_BUILT = {}


def _build():
    if "nc" in _BUILT:
        return _BUILT["nc"]
    nc = bacc.Bacc("TRN2", target_bir_lowering=False, debug=False)
    d = {}
    d["lt8"] = nc.dram_tensor("lt8", [N, N], FP8, kind="ExternalInput").ap()
    d["lres"] = nc.dram_tensor("lres", [N, N], FP8, kind="ExternalInput").ap()
    d["xq"] = nc.dram_tensor("xq", [2, N, C], FP8, kind="ExternalInput").ap()
    d["xt"] = nc.dram_tensor("xt", [C, N], BF16, kind="ExternalInput").ap()
    d["w8"] = nc.dram_tensor("w8", [128, 2, 512], FP8,
                             kind="ExternalInput").ap()
    d["wb"] = nc.dram_tensor("wb", [128, 8 * 128], BF16,
                             kind="ExternalInput").ap()
    d["bias"] = nc.dram_tensor("bias", [128, 6], F32,
                               kind="ExternalInput").ap()
    d["out"] = nc.dram_tensor("out", [T, C, N], F32, kind="ExternalOutput").ap()

    with tile.TileContext(nc) as tc, ExitStack() as ctx:
        _emit(ctx, tc, d)
    nc.compile()
    _BUILT["nc"] = nc
    return nc


def _bd(m):
    """[64,64] -> block-diagonal [128,128] (two independent batches)."""
    z = np.zeros((128, 128), np.float32)
    z[:64, :64] = m
    z[64:, 64:] = m
    return z


def _f8(a):
    return np.clip(np.asarray(a, np.float32), -240.0, 240.0).astype(NPF8)


def make_in_maps(inputs_edge, L_tilde, W_gate, b_gate, W_upd, b_upd,
                 W_edge, b_edge):
    """Host-side layout transforms, scaling and fp8/bf16 casts + sharding."""
    x = np.asarray(inputs_edge, np.float32)
    L = np.asarray(L_tilde, np.float32)
    Wg0, Wg1 = np.asarray(W_gate[0], np.float32), np.asarray(W_gate[1], np.float32)
    Wu0, Wu1 = np.asarray(W_upd[0], np.float32), np.asarray(W_upd[1], np.float32)
    We = np.asarray(W_edge, np.float32)
    bg = np.asarray(b_gate, np.float32)
    bu = np.asarray(b_upd, np.float32)
    be = np.asarray(b_edge, np.float32)

    lt32 = 32.0 * L.T
    lt8 = _f8(lt32)
    lres = _f8(lt32 - lt8.astype(np.float32))

    eye = np.eye(128, dtype=np.float32)
    w8 = np.stack([
        np.concatenate([4 * _bd(Wg0[F:, :F]), 4 * _bd(Wg0[F:, F:]),
                        4 * _bd(Wu0[F:]), eye], axis=1),
        np.concatenate([4 * _bd(Wg1[F:, :F]), 4 * _bd(Wg1[F:, F:]),
                        4 * _bd(Wu1[F:]), eye], axis=1),
    ], axis=1)                                             # [128, 2, 512]
    wbp = np.concatenate(
        [32 * _bd(Wg0[:F, :F]), 32 * _bd(Wg0[:F, F:]), 32 * _bd(Wu0[:F]),
         32 * _bd(Wg1[:F, :F]), 32 * _bd(Wg1[:F, F:]), 32 * _bd(Wu1[:F]),
         _bd(We), eye], axis=1)                            # [128, 1024]
    t2 = lambda v: np.tile(v, 2)
    biasp = np.stack([32 * t2(bg[:F]), 32 * t2(bg[F:]), 32 * t2(bu),
                      -t2(bg[F:]), t2(bu), t2(be)], axis=1)  # [128, 6]

    shared = {
        "lt8": np.ascontiguousarray(lt8),
        "lres": np.ascontiguousarray(lres),
        "w8": np.ascontiguousarray(_f8(w8)),
        "wb": np.ascontiguousarray(wbp.astype(NPBF)),
        "bias": np.ascontiguousarray(biasp.astype(np.float32)),
    }
    in_maps = []
    for core in range(NCORES):
        xs = x[core * BL:(core + 1) * BL]                  # [BL, N, F]
        xnat = np.ascontiguousarray(xs.transpose(1, 0, 2).reshape(N, C))
        xh = _f8(8.0 * xnat)
        xl = _f8(8.0 * xnat - xh.astype(np.float32))
        m = dict(shared)
        m["xq"] = np.ascontiguousarray(np.stack([xh, xl]))  # [2, N, C]
        m["xt"] = np.ascontiguousarray(
            xs.transpose(0, 2, 1).reshape(C, N).astype(NPBF))
        in_maps.append(m)
    return in_maps


def unshard(core_outs):
    """[NCORES][T, C, N] -> [T, B, N, F]"""
    arr = np.stack([np.asarray(o, np.float32) for o in core_outs])
    return np.ascontiguousarray(
        arr.reshape(NCORES, T, BL, F, N)
           .transpose(1, 0, 2, 4, 3)
           .reshape(T, B, N, F).astype(np.float32))


def run(in_maps, **kw):
    nc = _build()
    return run_bass_kernel_spmd(nc, in_maps, list(range(NCORES)), **kw)


def kernel(inputs_edge, L_tilde, W_gate, b_gate, W_upd, b_upd, W_edge, b_edge):
    in_maps = make_in_maps(inputs_edge, L_tilde, W_gate, b_gate,
                           W_upd, b_upd, W_edge, b_edge)
    res = run(in_maps)
    return unshard([res.results[c]["out"] for c in range(NCORES)])


# revision 35
# speedup vs baseline: 1.8410x; 1.0783x over previous
# Trainium2 Bass kernel for the Chebyshev-GCN GRU decoder (gnn_message_passing).
#
# Problem: B=16, N=2048, F=64, K=2 Chebyshev taps, T=8 decode steps.
#   per step: gates = cheb(L, [x, hx]) @ W_gate; r,u = sigmoid(gates)
#             cy = tanh(cheb(L, [x, r*hx]) @ W_upd); hy = u*hx + (1-u)*cy
#             yt = sigmoid(hy @ W_edge)
#
# v2 strategy (fp8 DoubleRow everywhere on the per-step critical path):
#  - Data-parallel over batch: 8 cores x 2 batches each; transposed layout
#    [c, n] with c = b*64+f (128 partitions), block-diagonal 128x128 weights.
#  - x-only terms are step-invariant: computed ONCE at startup into gate
#    consts (grc/guc/ccc), stored as fp8 hi+lo pairs at x32 scale so the
#    per-step const-add rides the fp8 DoubleRow matmul path.
#  - Precompute L@x runs in fp8 hi/lo split form (L8@xh + L8@xl + Lres@xh,
#    all DoubleRow) which matches bf16 accuracy without an 8MB bf16 L load.
#  - Per-step big matmuls L@hx, L@(r*hx): fp8e4m3 DoubleRow (2 k-tiles of
#    128 per instruction, 0.5 cyc/row): L^T stored x32 fp8, activations x8
#    fp8; psum carries x256, drained x(1/32) to fp8 x8 moving operands.
#  - Per-step feature matmuls: fp8 DoubleRow with moving pairs
#    (hxT8|lxh8) / (rhT8|Lrh8) and weight pairs (W0|W1)x4; gate consts
#    enter the same psum group via an (I|I) @ (hi|lo) DoubleRow pair.
#    psum = 32x gates; Act sigmoid/tanh unwinds with scale=1/32.
#  - Edge output projection stays bf16 (fp8 there costs too much accuracy).
#  - Elementwise in bf16 on DVE (4x TensorScalarPtr perf mode) + GPSIMD for
#    the SBUF->SBUF fp8 casts (no PSUM port on gpsimd); psum drains split
#    DVE/Act.
#
# kernel() takes FULL unsharded inputs, returns FULL [T, B, N, F] output.

import numpy as np
import ml_dtypes
from contextlib import ExitStack

import concourse.bass as bass
import concourse.tile as tile
from concourse import bacc, mybir
from concourse.bass_utils import run_bass_kernel_spmd

F32 = mybir.dt.float32
BF16 = mybir.dt.bfloat16
FP8 = mybir.dt.float8e4
U32 = mybir.dt.uint32
DR = mybir.MatmulPerfMode.DoubleRow
NPF8 = ml_dtypes.float8_e4m3
NPBF = ml_dtypes.bfloat16

B, N, F = 16, 2048, 64
T = 8
NCORES = 8
BL = B // NCORES          # batches per core (2)
C = BL * F                # 128 partitions in transposed layout
NT = N // 128             # 16 contraction tiles
NBLK = 4
BLK = N // NBLK           # 512

MU = mybir.AluOpType.mult
AD = mybir.AluOpType.add
SU = mybir.AluOpType.subtract


def bw(blk):
    return slice(blk * BLK, (blk + 1) * BLK)


def _emit(ctx: ExitStack, tc: tile.TileContext, d):
    nc = tc.nc
    AF = mybir.ActivationFunctionType

    consts = ctx.enter_context(tc.tile_pool(name="consts", bufs=1))
    big_ps = ctx.enter_context(tc.tile_pool(name="bigps", bufs=2, space="PSUM"))
    ru_ps = ctx.enter_context(tc.tile_pool(name="rups", bufs=2, space="PSUM"))
    tr_ps = ctx.enter_context(tc.tile_pool(name="trps", bufs=1, space="PSUM"))

    # ---- persistent SBUF tiles --------------------------------------
    w8 = consts.tile([128, 2, 512], FP8, tag="w8")
    wb = consts.tile([128, 8 * 128], BF16, tag="wb")
    bias = consts.tile([128, 6], F32, tag="bias")
    lt8 = consts.tile([128, NT, N], FP8, tag="lt8")
    s_sb = consts.tile([128, NT, 128], FP8, tag="s")
    mvA = consts.tile([128, 2, N], FP8, tag="mvA")     # hxT8 | lxh8
    mvB = consts.tile([128, 2, N], FP8, tag="mvB")     # rhT8 | Lrh8
    gc = consts.tile([128, 2, N], FP8, tag="gc")       # hi | lo (x32)
    gu = consts.tile([128, 2, N], FP8, tag="gu")
    cc = consts.tile([128, 2, N], FP8, tag="cc")
    hx_t = [consts.tile([128, N], BF16, tag=f"hx{i}", name=f"hx{i}")
            for i in range(2)]
    ruT = consts.tile([128, 2, N], BF16, tag="ruT")    # r | u
    cyt = consts.tile([128, N], BF16, tag="cyt")
    qtmp = consts.tile([128, N], BF16, tag="qtmp")
    ytt = consts.tile([128, N], F32, tag="ytt")

    ident8 = w8[:, 0, 384:512]
    ipair = w8[:, :, 384:512]
    w_gr = w8[:, :, 0:128]
    w_gu = w8[:, :, 128:256]
    w_cc = w8[:, :, 256:384]
    wbm = {k: wb[:, i * 128:(i + 1) * 128]
           for i, k in enumerate(["wx0r", "wx0u", "wxc0",
                                  "wx1r", "wx1u", "wxc1", "we", "identb"])}
    b_gr32 = bias[:, 0:1]
    b_gu32 = bias[:, 1:2]
    b_cc32 = bias[:, 2:3]
    b_ngu = bias[:, 3:4]
    b_cct = bias[:, 4:5]
    b_ee = bias[:, 5:6]
    rT = ruT[:, 0, :]
    uT = ruT[:, 1, :]

    # ---- static loads (small first; L chunks in consumption order) --
    nc.sync.dma_start(w8[:], d["w8"][:, :, :])
    nc.sync.dma_start(bias[:], d["bias"][:, :])

    def emit_mms(hyT, half):
        """half of y[t]'s projection: 2 blocks into one [128,1024] psum."""
        ep = ru_ps.tile([128, 2 * BLK], F32, tag="ru", name="ep")
        for j in range(2):
            blk = 2 * half + j
            nc.tensor.matmul(ep[:, j * BLK:(j + 1) * BLK], wbm["we"],
                             hyT[:, bw(blk)], start=True, stop=True)
        return ep

    def emit_act(t, half, ep):
        hw2 = slice(half * 2 * BLK, (half + 1) * 2 * BLK)
        nc.scalar.activation(ytt[:, hw2], ep[:], AF.Sigmoid,
                             bias=b_ee, scale=1.0)
        nc.sync.dma_start(d["out"][t, :, hw2], ytt[:, hw2])

    # ---- precompute -------------------------------------------------
    with tc.tile_pool(name="pre", bufs=1) as pre:
        lres = pre.tile([128, NT, N], FP8, tag="lres")
        xq = pre.tile([128, 2, NT, 128], FP8, tag="xq")   # xh | xl planes
        xt = pre.tile([128, N], BF16, tag="xt")
        lxp = pre.tile([128, N], BF16, tag="lxp")

        nc.sync.dma_start(
            xq[:], d["xq"].rearrange("l (a p) c -> p l a c", p=128))
        dlt = d["lt8"].rearrange("(a p) c -> p a c", p=128)
        dlr = d["lres"].rearrange("(a p) c -> p a c", p=128)
        nc.sync.dma_start(lt8[:, :, bw(0)], dlt[:, :, bw(0)])
        nc.sync.dma_start(lres[:, :, bw(0)], dlr[:, :, bw(0)])
        nc.sync.dma_start(lt8[:, :, bw(1)], dlt[:, :, bw(1)])
        nc.sync.dma_start(wb[:], d["wb"][:, :])
        nc.sync.dma_start(xt[:], d["xt"][:, :])
        nc.sync.dma_start(lres[:, :, bw(1)], dlr[:, :, bw(1)])
        for blk in (2, 3):
            nc.sync.dma_start(lt8[:, :, bw(blk)], dlt[:, :, bw(blk)])
            nc.sync.dma_start(lres[:, :, bw(blk)], dlr[:, :, bw(blk)])

        # Lx = L8@(xh+xl) + Lres@xh   (all DoubleRow, psum x256)
        for blk in range(NBLK):
            ps = big_ps.tile([128, BLK], F32, tag="big")
            k = 0
            for plane, lsb in ((0, lt8), (1, lt8), (0, lres)):
                for m in range(8):
                    nc.tensor.matmul(
                        ps[:], xq[:, plane, 2 * m:2 * m + 2, :],
                        lsb[:, 2 * m:2 * m + 2, bw(blk)],
                        start=(k == 0), stop=(k == 23), perf_mode=DR)
                    k += 1
            nc.vector.tensor_scalar(lxp[:, bw(blk)], ps[:],
                                    1.0 / 256.0, None, op0=MU)

        # gate/cand consts (psum x32 via x32 bf16 weights) + step-0 tail
        trt0 = tr_ps.tile([128, 4 * BLK], BF16, tag="tr")
        for blk in range(NBLK):
            pg = ru_ps.tile([128, 2 * BLK], F32, tag="ru")
            nc.tensor.matmul(pg[:, 0:BLK], wbm["wx0r"], xt[:, bw(blk)],
                             start=True, stop=False)
            nc.tensor.matmul(pg[:, 0:BLK], wbm["wx1r"], lxp[:, bw(blk)],
                             start=False, stop=True)
            nc.tensor.matmul(pg[:, BLK:], wbm["wx0u"], xt[:, bw(blk)],
                             start=True, stop=False)
            nc.tensor.matmul(pg[:, BLK:], wbm["wx1u"], lxp[:, bw(blk)],
                             start=False, stop=True)
            pc_t = ru_ps.tile([128, 2 * BLK], F32, tag="ru")
            pc = pc_t[:, 0:BLK]
            nc.tensor.matmul(pc, wbm["wxc0"], xt[:, bw(blk)],
                             start=True, stop=False)
            nc.tensor.matmul(pc, wbm["wxc1"], lxp[:, bw(blk)],
                             start=False, stop=True)
            # hi/lo splits (consts include bias*32)
            nc.scalar.activation(gc[:, 0, bw(blk)], pg[:, 0:BLK],
                                 AF.Identity, bias=b_gr32, scale=1.0)
            nc.scalar.activation(gu[:, 0, bw(blk)], pg[:, BLK:],
                                 AF.Identity, bias=b_gu32, scale=1.0)
            nc.scalar.activation(cc[:, 0, bw(blk)], pc,
                                 AF.Identity, bias=b_cc32, scale=1.0)
            nc.vector.scalar_tensor_tensor(gc[:, 1, bw(blk)], pg[:, 0:BLK],
                                           b_gr32, gc[:, 0, bw(blk)],
                                           op0=AD, op1=SU)
            nc.vector.scalar_tensor_tensor(gu[:, 1, bw(blk)], pg[:, BLK:],
                                           b_gu32, gu[:, 0, bw(blk)],
                                           op0=AD, op1=SU)
            nc.vector.scalar_tensor_tensor(cc[:, 1, bw(blk)], pc,
                                           b_cc32, cc[:, 0, bw(blk)],
                                           op0=AD, op1=SU)
            # step 0: ucompl = sigmoid(-gu0), cy0 = tanh(cc0),
            # hy0 = ucompl*cy0  (psums are x32, biases not yet applied)
            nc.scalar.activation(uT[:, bw(blk)], pg[:, BLK:], AF.Sigmoid,
                                 bias=b_ngu, scale=-1.0 / 32.0)
            nc.scalar.activation(cyt[:, bw(blk)], pc, AF.Tanh,
                                 bias=b_cct, scale=1.0 / 32.0)
            nc.vector.tensor_mul(hx_t[1][:, bw(blk)], uT[:, bw(blk)],
                                 cyt[:, bw(blk)])
            nc.scalar.mul(mvA[:, 0, bw(blk)], hx_t[1][:, bw(blk)], 8.0)
            for j in range(4):
                ti = blk * 4 + j
                nc.tensor.transpose(trt0[:, ti * 128:(ti + 1) * 128],
                                    hx_t[1][:, ti * 128:(ti + 1) * 128],
                                    wbm["identb"])
        for h in range(2):
            nc.vector.tensor_scalar(
                s_sb[:, h * 8:h * 8 + 8, :].rearrange("p a c -> p (a c)"),
                trt0[:, h * 1024:(h + 1) * 1024], 8.0, None, op0=MU)

    # ---- steps 1..T-1 ----------------------------------------------
    for t in range(1, T):
        hxT, hyT = hx_t[t % 2], hx_t[(t + 1) % 2]
        # Emission builds per-engine queues so no ready work sits behind
        # blocked work.  Big matmuls: per block, k-pairs 0-3 then 4-7 as
        # separate half-accumulations interleaved across a block pair.
        def half(ps, blk, mr):
            for m in mr:
                nc.tensor.matmul(ps[:], s_sb[:, 2 * m:2 * m + 2, :],
                                 lt8[:, 2 * m:2 * m + 2, bw(blk)],
                                 start=(m == 0), stop=(m == 7), perf_mode=DR)

        lo, hi = range(0, 4), range(4, 8)

        # --- A phase -------------------------------------------------
        # A1 group 0 (blocks 0,1) + gates01 + r-sigs, then group 1.
        pra = [None, None]
        pua = [None, None]

        def gates_blk(h, j):
            if j == 0:
                pra[h] = ru_ps.tile([128, 2 * BLK], F32, tag="ru", name="pra")
                pua[h] = ru_ps.tile([128, 2 * BLK], F32, tag="ru", name="pua")
            blk = 2 * h + j
            js = slice(j * BLK, (j + 1) * BLK)
            for pg, w_g, gk in ((pra[h], w_gr, gc), (pua[h], w_gu, gu)):
                nc.tensor.matmul(pg[:, js], w_g, mvA[:, :, bw(blk)],
                                 start=True, stop=False, perf_mode=DR)
                nc.tensor.matmul(pg[:, js], ipair, gk[:, :, bw(blk)],
                                 start=False, stop=True, perf_mode=DR)

        def r_sig(blk):
            nc.scalar.activation(rT[:, bw(blk)],
                                 pra[blk // 2][:, (blk % 2) * BLK:
                                               (blk % 2 + 1) * BLK],
                                 AF.Sigmoid, scale=1.0 / 32.0)
            nc.vector.scalar_tensor_tensor(mvB[:, 0, bw(blk)],
                                           rT[:, bw(blk)], 8.0,
                                           hxT[:, bw(blk)], op0=MU, op1=MU)

        bigp = {}
        for g in range(2):
            b0, b1 = 2 * g, 2 * g + 1
            ps0 = big_ps.tile([128, BLK], F32, tag="big", name="ps0")
            ps1 = big_ps.tile([128, BLK], F32, tag="big", name="ps1")
            half(ps0, b0, lo)
            half(ps1, b1, lo)
            half(ps0, b0, hi)
            half(ps1, b1, hi)
            for ps, blk in ((ps0, b0), (ps1, b1)):
                nc.scalar.mul(mvA[:, 1, bw(blk)], ps[:], 1.0 / 32.0)
            gates_blk(g, 0)
            gates_blk(g, 1)
            r_sig(2 * g)
            r_sig(2 * g + 1)

        def u_sig(blk):
            nc.scalar.activation(uT[:, bw(blk)],
                                 pua[blk // 2][:, (blk % 2) * BLK:
                                               (blk % 2 + 1) * BLK],
                                 AF.Sigmoid, scale=1.0 / 32.0)

        for blk in range(NBLK):
            u_sig(blk)

        # rh transposes (per block) with punned u32 drains; interleaved
        # with B1 half-accumulations so B1 never waits on late blocks.
        trt = tr_ps.tile([128, 4 * BLK], BF16, tag="tr")

        def tr_rh(blk):
            for j in range(4):
                ti = blk * 4 + j
                nc.tensor.transpose(trt[:, ti * 128:(ti + 1) * 128],
                                    rT[:, ti * 128:(ti + 1) * 128],
                                    wbm["identb"])
            sl = (s_sb[:, blk * 4:blk * 4 + 4, :]
                  .rearrange("p a c -> p (a c)"))
            nc.vector.tensor_mul(sl, sl, trt[:, bw(blk)])

        tr_rh(0)
        tr_rh(1)
        pb0 = big_ps.tile([128, BLK], F32, tag="big", name="pb0")
        pb1 = big_ps.tile([128, BLK], F32, tag="big", name="pb1")
        half(pb0, 0, lo)
        half(pb1, 1, lo)
        tr_rh(2)
        tr_rh(3)
        half(pb0, 0, hi)
        half(pb1, 1, hi)
        for ps, blk in ((pb0, 0), (pb1, 1)):
            nc.scalar.mul(mvB[:, 1, bw(blk)], ps[:], 1.0 / 32.0)
        # z = 1-u, W = u*hx ride the DVE hole while B1 group 1 runs
        for h in range(2):
            hs = slice(h * 1024, (h + 1) * 1024)
            nc.vector.tensor_scalar(qtmp[:, hs], uT[:, hs],
                                    -1.0, 1.0, op0=MU, op1=AD)
            nc.vector.tensor_mul(hyT[:, hs], uT[:, hs], hxT[:, hs])
        pb2 = big_ps.tile([128, BLK], F32, tag="big", name="pb2")
        pb3 = big_ps.tile([128, BLK], F32, tag="big", name="pb3")
        half(pb2, 2, lo)
        half(pb3, 3, lo)
        half(pb2, 2, hi)
        half(pb3, 3, hi)
        for ps, blk in ((pb2, 2), (pb3, 3)):
            nc.scalar.mul(mvB[:, 1, bw(blk)], ps[:], 1.0 / 32.0)

        # --- B tail: per-block cand -> tanh -> blend -> cast -> transpose
        trt2 = (tr_ps.tile([128, 4 * BLK], BF16, tag="tr", name="trt2")
                if t < T - 1 else None)
        pcs = [None, None]

        def cand_blk(blk):
            h, j = blk // 2, blk % 2
            if j == 0:
                pcs[h] = ru_ps.tile([128, 2 * BLK], F32, tag="ru", name="pc")
            js = slice(j * BLK, (j + 1) * BLK)
            nc.tensor.matmul(pcs[h][:, js], w_cc, mvB[:, :, bw(blk)],
                             start=True, stop=False, perf_mode=DR)
            nc.tensor.matmul(pcs[h][:, js], ipair, cc[:, :, bw(blk)],
                             start=False, stop=True, perf_mode=DR)

        def tanh_blk(blk):
            nc.scalar.activation(cyt[:, bw(blk)],
                                 pcs[blk // 2][:, (blk % 2) * BLK:
                                               (blk % 2 + 1) * BLK],
                                 AF.Tanh, scale=1.0 / 32.0)

        def blend_blk(blk):
            nc.vector.tensor_mul(qtmp[:, bw(blk)], qtmp[:, bw(blk)],
                                 cyt[:, bw(blk)])
            nc.vector.tensor_add(hyT[:, bw(blk)], hyT[:, bw(blk)],
                                 qtmp[:, bw(blk)])

        def cast_blk(blk):
            nc.scalar.mul(mvA[:, 0, bw(blk)], hyT[:, bw(blk)], 8.0)

        def tr_hy(blk):
            for j in range(4):
                ti = blk * 4 + j
                nc.tensor.transpose(trt2[:, ti * 128:(ti + 1) * 128],
                                    hyT[:, ti * 128:(ti + 1) * 128],
                                    wbm["identb"])
            dsl = (s_sb[:, blk * 4:blk * 4 + 4, :]
                   .rearrange("p a c -> p (a c)"))
            nc.vector.tensor_scalar(dsl, trt2[:, bw(blk)], 8.0, None,
                                    op0=MU)

        last = t == T - 1
        cand_blk(0)
        cand_blk(1)
        tanh_blk(0)
        cand_blk(2)
        tanh_blk(1)
        cand_blk(3)
        blend_blk(0)
        if not last:
            tr_hy(0)
        tanh_blk(2)
        blend_blk(1)
        if not last:
            tr_hy(1)
        tanh_blk(3)
        blend_blk(2)
        if not last:
            tr_hy(2)
        blend_blk(3)
        if not last:
            tr_hy(3)
        ep0 = emit_mms(hxT, 0)
        cast_blk(0)
        cast_blk(1)
        ep1 = emit_mms(hxT, 1)
        cast_blk(2)
        cast_blk(3)
        emit_act(t - 1, 0, ep0)
        emit_act(t - 1, 1, ep1)
        if t == T - 1:
            epa = emit_mms(hyT, 0)
            emit_act(t, 0, epa)
            epb = emit_mms(hyT, 1)
            emit_act(t, 1, epb)


_BUILT = {}


def _build():
    if "nc" in _BUILT:
        return _BUILT["nc"]
    nc = bacc.Bacc("TRN2", target_bir_lowering=False, debug=False)
    d = {}
    d["lt8"] = nc.dram_tensor("lt8", [N, N], FP8, kind="ExternalInput").ap()
    d["lres"] = nc.dram_tensor("lres", [N, N], FP8, kind="ExternalInput").ap()
    d["xq"] = nc.dram_tensor("xq", [2, N, C], FP8, kind="ExternalInput").ap()
    d["xt"] = nc.dram_tensor("xt", [C, N], BF16, kind="ExternalInput").ap()
    d["w8"] = nc.dram_tensor("w8", [128, 2, 512], FP8,
                             kind="ExternalInput").ap()
    d["wb"] = nc.dram_tensor("wb", [128, 8 * 128], BF16,
                             kind="ExternalInput").ap()
    d["bias"] = nc.dram_tensor("bias", [128, 6], F32,
                               kind="ExternalInput").ap()
    d["out"] = nc.dram_tensor("out", [T, C, N], F32, kind="ExternalOutput").ap()

    with tile.TileContext(nc) as tc, ExitStack() as ctx:
        _emit(ctx, tc, d)
    nc.compile()
    _BUILT["nc"] = nc
    return nc


def _bd(m):
    """[64,64] -> block-diagonal [128,128] (two independent batches)."""
    z = np.zeros((128, 128), np.float32)
    z[:64, :64] = m
    z[64:, 64:] = m
    return z


def _f8(a):
    return np.clip(np.asarray(a, np.float32), -240.0, 240.0).astype(NPF8)


def make_in_maps(inputs_edge, L_tilde, W_gate, b_gate, W_upd, b_upd,
                 W_edge, b_edge):
    """Host-side layout transforms, scaling and fp8/bf16 casts + sharding."""
    x = np.asarray(inputs_edge, np.float32)
    L = np.asarray(L_tilde, np.float32)
    Wg0, Wg1 = np.asarray(W_gate[0], np.float32), np.asarray(W_gate[1], np.float32)
    Wu0, Wu1 = np.asarray(W_upd[0], np.float32), np.asarray(W_upd[1], np.float32)
    We = np.asarray(W_edge, np.float32)
    bg = np.asarray(b_gate, np.float32)
    bu = np.asarray(b_upd, np.float32)
    be = np.asarray(b_edge, np.float32)

    lt32 = 32.0 * L.T
    lt8 = _f8(lt32)
    lres = _f8(lt32 - lt8.astype(np.float32))

    eye = np.eye(128, dtype=np.float32)
    w8 = np.stack([
        np.concatenate([4 * _bd(Wg0[F:, :F]), 4 * _bd(Wg0[F:, F:]),
                        4 * _bd(Wu0[F:]), eye], axis=1),
        np.concatenate([4 * _bd(Wg1[F:, :F]), 4 * _bd(Wg1[F:, F:]),
                        4 * _bd(Wu1[F:]), eye], axis=1),
    ], axis=1)                                             # [128, 2, 512]
    wbp = np.concatenate(
        [32 * _bd(Wg0[:F, :F]), 32 * _bd(Wg0[:F, F:]), 32 * _bd(Wu0[:F]),
         32 * _bd(Wg1[:F, :F]), 32 * _bd(Wg1[:F, F:]), 32 * _bd(Wu1[:F]),
         _bd(We), eye], axis=1)                            # [128, 1024]
    t2 = lambda v: np.tile(v, 2)
    biasp = np.stack([32 * t2(bg[:F]), 32 * t2(bg[F:]), 32 * t2(bu),
                      -t2(bg[F:]), t2(bu), t2(be)], axis=1)  # [128, 6]

    shared = {
        "lt8": np.ascontiguousarray(lt8),
        "lres": np.ascontiguousarray(lres),
        "w8": np.ascontiguousarray(_f8(w8)),
        "wb": np.ascontiguousarray(wbp.astype(NPBF)),
        "bias": np.ascontiguousarray(biasp.astype(np.float32)),
    }
    in_maps = []
    for core in range(NCORES):
        xs = x[core * BL:(core + 1) * BL]                  # [BL, N, F]
        xnat = np.ascontiguousarray(xs.transpose(1, 0, 2).reshape(N, C))
        xh = _f8(8.0 * xnat)
        xl = _f8(8.0 * xnat - xh.astype(np.float32))
        m = dict(shared)
        m["xq"] = np.ascontiguousarray(np.stack([xh, xl]))  # [2, N, C]
        m["xt"] = np.ascontiguousarray(
            xs.transpose(0, 2, 1).reshape(C, N).astype(NPBF))
        in_maps.append(m)
    return in_maps


def unshard(core_outs):
    """[NCORES][T, C, N] -> [T, B, N, F]"""
    arr = np.stack([np.asarray(o, np.float32) for o in core_outs])
    return np.ascontiguousarray(
        arr.reshape(NCORES, T, BL, F, N)
           .transpose(1, 0, 2, 4, 3)
           .reshape(T, B, N, F).astype(np.float32))


def run(in_maps, **kw):
    nc = _build()
    return run_bass_kernel_spmd(nc, in_maps, list(range(NCORES)), **kw)


def kernel(inputs_edge, L_tilde, W_gate, b_gate, W_upd, b_upd, W_edge, b_edge):
    in_maps = make_in_maps(inputs_edge, L_tilde, W_gate, b_gate,
                           W_upd, b_upd, W_edge, b_edge)
    res = run(in_maps)
    return unshard([res.results[c]["out"] for c in range(NCORES)])
